# revision 18
# baseline (speedup 1.0000x reference)
"""Grouped-experts MoE FFN (SwiGLU) kernel for Trainium2, expert-parallel on 8 cores.

E=8 experts, D=2048, H=5632, T=32768 tokens pre-sorted by expert.
Each NeuronCore owns one expert and its token shard (padded to 4096 tokens).

Phase 1 (x@w1T, x@w3T) uses a 1-level Strassen-Winograd block decomposition of
the stacked GEMM [W1;W3] @ X^T with 2x2 blocking over (D, T): 7 products
instead of 8 -> 12.5% fewer PE cycles.  All weight-side operands (A11, A12,
S1..S4, -A22) are precomputed on the host; the token-side operands (B11, B21,
B22, T1..T4) are precomputed on the host from the bf16-rounded x and stored as
fp16 (exact for sums of bf16 values; fp16 runs at full 1 cyc/row PE rate).
The product combinations (all '+' after host-negating A22) run on the idle
Vector engine, reading PSUM.  Extra error vs the bf16 reference ~0.04%.

Phase 2 (h@w2T) is the baseline scheme: the first NDR of 44 h-tiles go through
the w2 GEMM in fp8e4 DoubleRow mode (2 k-tiles per instruction).  Scale
folding keeps one PSUM accumulation: h-fp8 is stored as h*64 (applied by the
DVE scalar_tensor_tensor that builds h), w2-fp8 rows are *64, and the bf16 w2
rows are *4096, so every contribution carries 4096x; the final copy divides by
4096 (exact powers of two).

Token blocks: 4 paired blocks of 512+512 tokens (one 512-chunk from each
T-half, as required by the Strassen T-split); h stays resident in SBUF per
block.
"""

import sys

sys.path.insert(0, "/opt/trn_rl_repo")

import ml_dtypes
import numpy as np

import concourse.bass as bass  # noqa: F401
import concourse.mybir as mybir
import concourse.tile as tile
from concourse import bacc
from concourse.bass_utils import run_bass_kernel_spmd

BF16 = ml_dtypes.bfloat16
FP16 = np.float16
FP8 = ml_dtypes.float8_e4m3  # trn2 float8e4 (IEEE-ish, max 240)

E, D, H, T = 8, 2048, 5632, 32768
N_CORES = 8
TPC = T // E  # tokens per core (4096), also the padded shard size
TH = TPC // 2  # tokens per half (Strassen T-split) = 2048
D2 = D // 2  # Strassen D-split = 1024

NDR = 10  # h-tiles (of 44) routed through fp8 DoubleRow in phase 2; even
SH = 64.0  # fp8 h scale (applied on the fly by the h-producing DVE op)
SW = 64.0  # fp8 w2 scale
SPROD = SH * SW  # product scale carried by every PSUM contribution

TC = 512  # token chunk per half; block = 2*TC tokens
NBLK = TH // TC  # 4 paired blocks


def _build(d=D, h=H, tpc=TPC, tc=TC, ndr=NDR):
    """Build the Bass program (same program for all 8 cores; data differs)."""
    kd2 = D2 // 128  # 8 k-tiles per Strassen product
    kh = h // 128  # 44
    npair = ndr // 2
    nc = bacc.Bacc("TRN2", target_bir_lowering=False, debug=False)

    f32 = mybir.dt.float32
    bf16 = mybir.dt.bfloat16
    f16 = mybir.dt.float16
    fp8 = mybir.dt.float8e4
    SILU = mybir.ActivationFunctionType.Silu
    DR = mybir.MatmulPerfMode.DoubleRow
    MUL = mybir.AluOpType.mult

    # phase-1 Strassen operands (host-prepped, fp16):
    #   xop[o, p, ki, t] = Bop_o[ki*128+p, t]   (7 ops x [D2 x TH])
    xop = nc.dram_tensor("xop", [7, 128, kd2, TH], f16, kind="ExternalInput")
    #   aop[o, hi, p, ki, c] = Aop_o[hi*128+c, ki*128+p]  (7 ops x [H x D2])
    aop = nc.dram_tensor("aop", [7, kh, 128, kd2, 128], f16, kind="ExternalInput")
    # phase-2 weights (baseline layout):
    #   w2t[di, p, hk, c] = w2.T[hk*128+p, di*128+c] * 4096  (bf16 rows ndr..43)
    w2t = nc.dram_tensor("w2t", [d // 128, 128, kh, 128], bf16, kind="ExternalInput")
    outT = nc.dram_tensor("outT", [d, tpc], bf16, kind="ExternalOutput")
    if ndr:
        # w2f[di, p, pair, j, c] = w2.T[(2*pair+j)*128+p, di*128+c] * 64  (fp8)
        w2f = nc.dram_tensor("w2f", [d // 128, 128, npair, 2, 128], fp8, kind="ExternalInput")

    outr = outT.rearrange("(k p) t -> p k t", p=128)

    # product issue order and which xop each Aop pairs with:
    #   P1=A11*B11  P2=A12*B21  P3=S4*B22  P4n=(-A22)*T4  P5=S1*T1  P6=S2*T2  P7=S3*T3
    # op index o: 0:A11/B11 1:A12/B21 2:S4/B22 3:-A22/T4 4:S1/T1 5:S2/T2 6:S3/T3
    with tile.TileContext(nc) as tcx:
        with (
            tcx.tile_pool(name="sx", bufs=1) as sx,
            tcx.tile_pool(name="swa", bufs=2) as swa,
            tcx.tile_pool(name="stmp", bufs=2) as stmp,
            tcx.tile_pool(name="ssil", bufs=2) as ssil,
            tcx.tile_pool(name="sh", bufs=2 * (kh - ndr)) as sh,
            tcx.tile_pool(name="shf", bufs=max(npair, 1)) as shf,
            tcx.tile_pool(name="sw2", bufs=2) as sw2,
            tcx.tile_pool(name="sout", bufs=2) as sout,
            tcx.tile_pool(name="psP", bufs=5, space="PSUM") as psP,
            tcx.tile_pool(name="pso", bufs=2, space="PSUM") as pso,
        ):
            for b in range(NBLK):
                t0 = b * tc  # within-half start
                # prologue: for block 0, pull hi=0's weight tiles ahead of the
                # bulk x DMAs so the first matmul chain starts ~20us earlier
                pre_a = None
                if b == 0:
                    # hi=0 weight tiles on the ScalarE-issued DMA ring so they
                    # land in parallel with the x tiles on the sync ring
                    pre_a = {}
                    for o in (0, 5, 6, 1, 3, 4, 2):
                        pre_a[o] = swa.tile(
                            [128, kd2, 128], f16, tag=f"a{o}", bufs=2, name=f"a{o}_{b}_0"
                        )
                        nc.scalar.dma_start(pre_a[o][:], aop[o, 0])
                # ---- x operand tiles for this block (single-buffered; the
                # DMA for block b+1 overlaps phase 2 of block b) ----
                x_sb = {}
                for o in (0, 5, 6, 1, 3, 4, 2):  # first-needed first
                    x_sb[o] = sx.tile([128, kd2, tc], f16, tag=f"x{o}", bufs=1, name=f"x{o}_{b}")
                    if b == 0:
                        # two halves so the first product chain starts after
                        # only half the tile has landed
                        nc.sync.dma_start(x_sb[o][:, : kd2 // 2, :], xop[o, :, : kd2 // 2, t0 : t0 + tc])
                        nc.sync.dma_start(x_sb[o][:, kd2 // 2 :, :], xop[o, :, kd2 // 2 :, t0 : t0 + tc])
                    else:
                        nc.sync.dma_start(x_sb[o][:], xop[o, :, :, t0 : t0 + tc])

                # ---- phase 1: Strassen-Winograd products + combines ----
                h_tiles = {}  # (hi, half) -> bf16 [128, tc] for hi >= ndr
                hf_tiles = []  # per pair: fp8 [128, 2(hj), 2(half), tc], h*64
                for hp in range(kh // 2):
                    if 2 * hp < ndr:
                        hf_sb = shf.tile(
                            [128, 2, 2, tc], fp8, tag="hf", bufs=max(npair, 1), name=f"hf_{b}_{hp}"
                        )
                        hf_tiles.append(hf_sb)
                    for hj in range(2):
                        hi = hp * 2 + hj
                        if pre_a is not None and hi == 0:
                            a_sb = pre_a
                        else:
                            a_sb = {}
                            for o in (0, 5, 6, 1, 3, 4, 2):
                                a_sb[o] = swa.tile(
                                    [128, kd2, 128], f16, tag=f"a{o}", bufs=2, name=f"a{o}_{b}_{hi}"
                                )
                                nc.sync.dma_start(a_sb[o][:], aop[o, hi])

                        def prod(o, nm):
                            ps = psP.tile([128, tc], f32, tag="P", bufs=5, name=f"ps{nm}_{b}_{hi}")
                            for ki in range(kd2):
                                nc.tensor.matmul(
                                    ps[:],
                                    a_sb[o][:, ki, :],
                                    x_sb[o][:, ki, :],
                                    start=(ki == 0),
                                    stop=(ki == kd2 - 1),
                                )
                            return ps

                        # products interleaved with their DVE consumers
                        p1 = prod(0, "1")
                        p1s = stmp.tile([128, tc], f32, tag="p1s", bufs=2, name=f"p1s_{b}_{hi}")
                        nc.scalar.copy(p1s[:], p1[:])  # shared product -> SBUF
                        p6 = prod(5, "6")
                        u2 = stmp.tile([128, tc], f32, tag="u2", bufs=2, name=f"u2_{b}_{hi}")
                        nc.vector.tensor_add(u2[:], p1s[:], p6[:])
                        p7 = prod(6, "7")
                        u3 = stmp.tile([128, tc], f32, tag="u3", bufs=2, name=f"u3_{b}_{hi}")
                        nc.vector.tensor_add(u3[:], u2[:], p7[:])
                        p2 = prod(1, "2")
                        c11 = stmp.tile([128, tc], bf16, tag="ch1", bufs=2, name=f"c11_{b}_{hi}")
                        nc.vector.tensor_add(c11[:], p1s[:], p2[:])  # h1 half-0
                        sil1 = ssil.tile([128, tc], bf16, tag="sil", bufs=2, name=f"sil1_{b}_{hi}")
                        nc.scalar.activation(sil1[:], c11[:], SILU)
                        p4n = prod(3, "4")
                        c21 = stmp.tile([128, tc], bf16, tag="ch3", bufs=2, name=f"c21_{b}_{hi}")
                        nc.vector.tensor_add(c21[:], u3[:], p4n[:])  # h3 half-0
                        if hi < ndr:
                            nc.vector.scalar_tensor_tensor(
                                hf_tiles[hp][:, hj, 0, :], c21[:], SH, sil1[:], MUL, MUL
                            )
                        else:
                            h_a = sh.tile(
                                [128, tc], bf16, tag="h", bufs=2 * (kh - ndr), name=f"h_{b}_{hi}_0"
                            )
                            nc.vector.tensor_mul(h_a[:], sil1[:], c21[:])
                            h_tiles[(hi, 0)] = h_a
                        p5 = prod(4, "5")
                        c22 = stmp.tile([128, tc], bf16, tag="ch3", bufs=2, name=f"c22_{b}_{hi}")
                        nc.vector.tensor_add(c22[:], u3[:], p5[:])  # h3 half-1
                        p3 = prod(2, "3")
                        t1 = stmp.tile([128, tc], f32, tag="t1", bufs=2, name=f"t1_{b}_{hi}")
                        nc.vector.tensor_add(t1[:], u2[:], p5[:])
                        c12 = stmp.tile([128, tc], bf16, tag="ch1", bufs=2, name=f"c12_{b}_{hi}")
                        nc.vector.tensor_add(c12[:], t1[:], p3[:])  # h1 half-1
                        sil2 = ssil.tile([128, tc], bf16, tag="sil", bufs=2, name=f"sil2_{b}_{hi}")
                        nc.scalar.activation(sil2[:], c12[:], SILU)
                        if hi < ndr:
                            nc.vector.scalar_tensor_tensor(
                                hf_tiles[hp][:, hj, 1, :], c22[:], SH, sil2[:], MUL, MUL
                            )
                        else:
                            h_b = sh.tile(
                                [128, tc], bf16, tag="h", bufs=2 * (kh - ndr), name=f"h_{b}_{hi}_1"
                            )
                            nc.vector.tensor_mul(h_b[:], sil2[:], c22[:])
                            h_tiles[(hi, 1)] = h_b

                # ---- phase 2: outT = (h_fp8 @ w2f.T + h @ w2.T) / 4096 ----
                for di in range(d // 128):
                    w2_sb = sw2.tile([128, kh - ndr, 128], bf16, tag="w2", bufs=2, name=f"w2_{b}_{di}")
                    nc.sync.dma_start(w2_sb[:], w2t[di, :, ndr:, :])
                    if ndr:
                        w2f_sb = sw2.tile([128, npair, 2, 128], fp8, tag="w2f", bufs=2, name=f"w2f_{b}_{di}")
                        nc.sync.dma_start(w2f_sb[:], w2f[di])
                    # both halves grouped per di: only ONE bf16->DR transition
                    # per di (each exposes ~190ns of un-hidden DR LDWEIGHTS),
                    # and consecutive same-weight MMs for halves 0/1
                    ps_h = [
                        pso.tile([128, tc], f32, tag="o", bufs=3, name=f"pso_{b}_{di}_{half}")
                        for half in range(2)
                    ]
                    for hk in range(ndr, kh):
                        for half in range(2):
                            nc.tensor.matmul(
                                ps_h[half][:],
                                w2_sb[:, hk - ndr, :],
                                h_tiles[(hk, half)][:],
                                start=(hk == ndr),
                                stop=(ndr == 0 and hk == kh - 1),
                            )
                    for pair in range(npair):
                        for half in range(2):
                            nc.tensor.matmul(
                                ps_h[half][:],
                                w2f_sb[:, pair],
                                hf_tiles[pair][:, :, half, :],
                                start=False,
                                stop=(pair == npair - 1),
                                perf_mode=DR,
                            )
                    for half in range(2):
                        tg = t0 + half * TH  # global token col
                        o_sb = sout.tile([128, tc], bf16, tag="osb", bufs=2, name=f"o_{b}_{di}_{half}")
                        nc.scalar.mul(o_sb[:], ps_h[half][:], 1.0 / SPROD)
                        nc.sync.dma_start(outr[:, di, tg : tg + tc], o_sb[:])
    nc.compile()
    return nc


_NC = None


def _get_nc():
    global _NC
    if _NC is None:
        _NC = _build()
    return _NC


def _prep_core(args):
    """Host-side shard prep for one expert: slice+pad tokens, Strassen operands."""
    x, w1, w3, w2, off, cnt = args
    xe = np.zeros((TPC, D), dtype=BF16)
    xe[:cnt] = x[off : off + cnt].astype(BF16)
    Xt = np.ascontiguousarray(xe.T).astype(np.float32)  # [D, TPC], bf16 values
    B11 = Xt[:D2, :TH]
    B12 = Xt[:D2, TH:]
    B21 = Xt[D2:, :TH]
    B22 = Xt[D2:, TH:]
    T1 = B12 - B11
    T2 = B22 - T1
    T3 = B22 - B12
    T4 = T2 - B21
    xops = np.stack([B11, B21, B22, T4, T1, T2, T3]).astype(FP16)  # [7, D2, TH]
    xop = np.ascontiguousarray(xops.reshape(7, D2 // 128, 128, TH).transpose(0, 2, 1, 3))

    w1b = w1.astype(BF16).astype(np.float32)  # [H, D]
    w3b = w3.astype(BF16).astype(np.float32)
    A11, A12 = w1b[:, :D2], w1b[:, D2:]
    A21, A22 = w3b[:, :D2], w3b[:, D2:]
    S1 = A21 + A22
    S2 = S1 - A11
    S3 = A11 - A21
    S4 = A12 - S2
    wops = np.stack([A11, A12, S4, -A22, S1, S2, S3]).astype(FP16)  # [7, H, D2]
    aopa = np.ascontiguousarray(
        wops.reshape(7, H // 128, 128, D2 // 128, 128).transpose(0, 1, 4, 3, 2)
    )

    # w2 [D, H] -> [di, p, hk, c] with w2t[di,p,hk,c] = w2[di*128+c, hk*128+p]
    w2p = np.ascontiguousarray(
        (w2 * SPROD).astype(BF16).reshape(D // 128, 128, H // 128, 128).transpose(0, 3, 2, 1)
    )
    out = {
        "xop": xop,
        "aop": aopa,
        "w2t": w2p,
    }
    if NDR:
        # fp8 rows: [di, p, pair, j, c] = w2[di*128+c, (2*pair+j)*128+p] * 64
        w2cols = (w2[:, : NDR * 128] * SW).astype(FP8)
        out["w2f"] = np.ascontiguousarray(
            w2cols.reshape(D // 128, 128, NDR // 2, 2, 128).transpose(0, 4, 2, 3, 1)
        )
    return out


def kernel(x, w1, w2, w3, num_tokens_per_expert):
    x = np.asarray(x, dtype=np.float32)
    w1 = np.asarray(w1, dtype=np.float32)
    w2 = np.asarray(w2, dtype=np.float32)
    w3 = np.asarray(w3, dtype=np.float32)
    counts = np.asarray(num_tokens_per_expert).astype(np.int64)
    assert counts.shape == (E,) and counts.sum() == x.shape[0]
    assert counts.max() <= TPC, "per-expert shard exceeds compiled capacity"
    offs = np.concatenate([[0], np.cumsum(counts)[:-1]])

    from concurrent.futures import ThreadPoolExecutor

    with ThreadPoolExecutor(max_workers=8) as ex:
        in_maps = list(
            ex.map(
                _prep_core,
                [(x, w1[e], w3[e], w2[e], offs[e], counts[e]) for e in range(E)],
            )
        )

    nc = _get_nc()
    res = run_bass_kernel_spmd(nc, in_maps, core_ids=list(range(N_CORES)))

    out = np.empty((T, D), dtype=np.float32)

    def _post(e):
        oT = res.results[e]["outT"]  # [D, TPC] bf16
        out[offs[e] : offs[e] + counts[e]] = oT.T[: counts[e]].astype(np.float32)

    with ThreadPoolExecutor(max_workers=8) as ex:
        list(ex.map(_post, range(E)))
    return out
